# revision 9
# baseline (speedup 1.0000x reference)
"""AdaptiveLinearWithChannel: per-channel complex matmul with hypernet rank-2
residual, sharded channel-parallel across 8 TRN2 NeuronCores.

out[c] = x[c] @ (W[model_idx,c] + u_c v_c^T) + bias[model_idx,c] + hyper_shift[c]
  x: (C=32, P=8192, D=128) complex; W_eff: (C, D, D) complex.

Host: tiny hypernet MLPs (1->10->10->{8D,2D}) + rank-2 residual -> W_eff
(float64), then cast x to bf16 and pre-transpose to (C, D, P) so the device
needs no on-chip transposes and half the DMA bytes. The combined shift
(bias + hyper_shift, a per-(c,d) constant) is added on HOST after readback --
it is tiny (C x D) and removing it from the device turns the PSUM->SBUF
epilogue into a pure copy.

Device (per core, 4 channels): for each 128-row chunk of x, two accumulating
bf16 matmuls with stationary xT chunks and column-interleaved moving operands
(Wr_0,Wi_0,...) and (-Wi_0,Wr_0,...), N=256 -> psum holds the complex-
interleaved output directly. The epilogue copies PSUM->SBUF in [128,8,256]
tiles, ALTERNATING between the DVE (vector) and ACT (scalar) engines so
neither engine serializes the tail. Loads are 2MB half-channel slabs on the
sync HWDGE queue; stores are 0.5MB quarters on the scalar HWDGE queue with a
partition-major DRAM layout (8KB contiguous runs). Host widens bf16 ->
complex64, restores row order, and adds the shift. DMA-bound: ~33.8MB/core.
"""

import sys

sys.path.insert(0, "/opt/trn_rl_repo")

import numpy as np

C, P, D = 32, 8192, 128
N_CORES = 8
CH = C // N_CORES  # channels per core
PSUB = 4096        # p-columns per DMA slab (2MB)
NSLAB = P // PSUB  # slabs per channel
NCHUNK = PSUB // 128  # 128-row chunks per slab (32)
NB = 4             # 128-chunks batched per PSUM tile / epilogue copy
SQ = 16            # chunks per output store (0.5MB)

_NC_CACHE = {}


def _build_nc():
    from concourse import bacc, mybir
    from concourse.tile import TileContext

    f32 = mybir.dt.float32
    bf16 = mybir.dt.bfloat16

    nc = bacc.Bacc()
    # x_real/x_imag combined: (c, d, 0, p)=re, (c, d, 1, p)=im -> one DMA/slab
    xt = nc.declare_dram_parameter("xt", [CH, D, 2, P], bf16, isOutput=False)
    # moving operand, host-interleaved to the exact SBUF layout:
    # wmov cols (Wr_0, Wi_0, ...); the partner (-Wi_0, Wr_0, ...) is derived
    # on-device (DVE is idle at start, saves 0.26MB of DMA)
    wmov = nc.declare_dram_parameter("wmov", [D, CH, 2 * D], bf16, isOutput=False)
    # partition-major output layout: (c, p128, k, 2D) -> 8KB contiguous runs
    # per partition on each quarter store; host restores row order.
    out = nc.declare_dram_parameter(
        "out", [CH, 128, NSLAB * NCHUNK, 2 * D], bf16, isOutput=True
    )

    with TileContext(nc) as tc:
        with (
            tc.tile_pool(name="const", bufs=1) as cpool,
            tc.tile_pool(name="xin", bufs=3) as xpool,
            tc.tile_pool(name="pop", bufs=4, space="PSUM") as popool,
            tc.tile_pool(name="oout", bufs=2) as opool,
        ):
            # weights on the scalar HWDGE queue (idle at start; the sync
            # queue begins streaming x slabs immediately)
            w_bf = cpool.tile([128, CH, 2 * D], bf16, tag="wbf")
            nc.scalar.dma_start(out=w_bf[:], in_=wmov[:])
            w_ng = cpool.tile([128, CH, 2 * D], bf16, tag="wng")
            nc.vector.tensor_scalar_mul(
                w_ng[:, :, 0 : 2 * D : 2], w_bf[:, :, 1 : 2 * D : 2], -1.0
            )
            nc.vector.tensor_copy(
                w_ng[:, :, 1 : 2 * D : 2], w_bf[:, :, 0 : 2 * D : 2]
            )

            tile_idx = 0
            for c in range(CH):
                w_r_slice = w_bf[:, c, :]
                w_i_slice = w_ng[:, c, :]
                for s in range(NSLAB):
                    x_slab = xpool.tile([128, 2, PSUB], bf16, tag="xri")
                    nc.sync.dma_start(
                        out=x_slab[:], in_=xt[c, :, :, s * PSUB : (s + 1) * PSUB]
                    )
                    out_sb = opool.tile([128, NCHUNK, 2 * D], bf16, tag="osb")
                    for t0 in range(0, NCHUNK, NB):
                        po = popool.tile([128, NB, 2 * D], f32, tag="po")
                        for b in range(NB):
                            k = t0 + b
                            nc.tensor.matmul(
                                po[:, b, :],
                                x_slab[:, 0, k * 128 : (k + 1) * 128],
                                w_r_slice,
                                start=True,
                                stop=False,
                            )
                            nc.tensor.matmul(
                                po[:, b, :],
                                x_slab[:, 1, k * 128 : (k + 1) * 128],
                                w_i_slice,
                                start=False,
                                stop=True,
                            )
                        # epilogue: pure PSUM->SBUF copy (shift added on
                        # host); alternate engines so neither serializes
                        dst = out_sb[:, t0 : t0 + NB, :]
                        if tile_idx % 2 == 0:
                            nc.vector.tensor_copy(dst, po[:, :, :])
                        else:
                            nc.scalar.copy(dst, po[:, :, :])
                        tile_idx += 1
                        # store each finished 16-chunk quarter (0.5MB); the
                        # very last slab stores per-tile (0.25MB) so the
                        # final store isn't serialized behind two copies
                        last = c == CH - 1 and s == NSLAB - 1
                        sq = NB if last else SQ
                        if (t0 + NB) % sq == 0:
                            q0 = t0 + NB - sq
                            kg = s * NCHUNK + q0
                            nc.scalar.dma_start(
                                out=out[c, :, kg : kg + sq, :],
                                in_=out_sb[:, q0 : q0 + sq, :],
                            )
    nc.compile()
    return nc


def _host_prep(inputs):
    """Hypernet MLPs + rank-2 residual on host (float64), -> per-core arrays."""
    import ml_dtypes

    bf16 = ml_dtypes.bfloat16

    def relu(a):
        return np.maximum(a, 0.0)

    t = np.asarray(inputs["t"], np.float64)  # (1, 1)
    idx = np.asarray(inputs["indices"])

    def hyper(W1, b1, W2, b2, W3, b3):
        W1, b1, W2, b2, W3, b3 = (
            np.asarray(p, np.float64)[idx] for p in (W1, b1, W2, b2, W3, b3)
        )
        h = relu(np.einsum("ti,cio->cto", t, W1) + b1[:, None, :])
        h = relu(np.einsum("cti,cio->cto", h, W2) + b2[:, None, :])
        return np.einsum("cti,cio->cto", h, W3) + b3[:, None, :]

    uv = hyper(*(inputs[k] for k in ("gW1", "gb1", "gW2", "gb2", "gW3", "gb3")))
    uv = uv[:, 0, :]  # (C, 8D)  (nt == 1)
    u = (uv[:, : 2 * D] + 1j * uv[:, 2 * D : 4 * D]).reshape(C, D, 2)
    v = (uv[:, 4 * D : 6 * D] + 1j * uv[:, 6 * D :]).reshape(C, D, 2)
    residual = u @ np.swapaxes(v, -1, -2)  # (C, D, D)

    mi = int(np.asarray(inputs["model_idx"]))
    weight = np.asarray(inputs["weight"], np.float64)
    bias = np.asarray(inputs["bias"], np.float64)
    w = weight[mi, ..., 0] + 1j * weight[mi, ..., 1]  # (C, D, D)
    b = bias[mi, ..., 0] + 1j * bias[mi, ..., 1]  # (C, 1, D)

    W_eff = w + residual  # (C, D, D)

    hs = hyper(*(inputs[k] for k in ("sW1", "sb1", "sW2", "sb2", "sW3", "sb3")))
    hs = hs[:, 0, :]  # (C, 2D)
    shift = b[:, 0, :] + (hs[:, :D] + 1j * hs[:, D:])  # (C, D), added on host

    Wr = W_eff.real.astype(np.float32)
    Wi = W_eff.imag.astype(np.float32)

    # moving operand with interleaved columns, partition(d)-major
    wmov = np.empty((C, D, 2 * D), np.float32)
    wmov[:, :, 0::2] = Wr
    wmov[:, :, 1::2] = Wi
    wmov = wmov.astype(bf16)  # (C, D, 2D)

    # x: cast to bf16 (RNE) and transpose to (C, D, 2, P) -- device needs no
    # on-chip transposes, and real/imag combine into one DMA per slab
    xt = np.empty((C, D, 2, P), bf16)
    xt[:, :, 0, :] = np.asarray(inputs["x_real"], np.float32).transpose(0, 2, 1)
    xt[:, :, 1, :] = np.asarray(inputs["x_imag"], np.float32).transpose(0, 2, 1)

    in_maps = []
    for core in range(N_CORES):
        c0 = core * CH
        in_maps.append(
            {
                "xt": xt[c0 : c0 + CH],
                # (CH,D,2D) -> (D,CH,2D)
                "wmov": np.ascontiguousarray(
                    wmov[c0 : c0 + CH].transpose(1, 0, 2)
                ),
            }
        )
    return in_maps, shift.astype(np.complex64)


def _assemble(outs, shift):
    """bf16 (CH, 128, 64, 2D) per core -> (1, C, P, D) complex64, + shift."""
    full = np.concatenate(outs, axis=0)  # (C, 128, 64, 2D) bf16
    # (c, p128, k, n) -> (c, k, p128, n): row p = k*128 + p128
    full = full.transpose(0, 2, 1, 3).reshape(C, P, 2 * D)
    u32 = full.view(np.uint16).astype(np.uint32) << 16
    f32 = u32.view(np.float32)
    res = np.ascontiguousarray(f32).view(np.complex64)  # (C, P, D)
    res += shift[:, None, :]
    return res[None]


def _get_nc():
    if "nc" not in _NC_CACHE:
        _NC_CACHE["nc"] = _build_nc()
    return _NC_CACHE["nc"]


def kernel(**inputs):
    from concourse.bass_utils import run_bass_kernel_spmd

    nc = _get_nc()
    in_maps, shift = _host_prep(inputs)
    res = run_bass_kernel_spmd(nc, in_maps, core_ids=list(range(N_CORES)))
    return _assemble([res.results[i]["out"] for i in range(N_CORES)], shift)
